# revision 53
# baseline (speedup 1.0000x reference)
"""Trainium2 Bass kernel for nn_DimNet (4D-conv net + pixel shuffle).

Math: the three 4D convs collapse to 2D convs over flattened angular dims:
  conv1:  in [25, 104, 104] -> out [400, 96, 96], 9x9 kernel
  conv2a: in [25, 104, 104] -> buf [180, 100, 100] (20ch x 3x3 angular window, 5x5)
  conv2b: buf [180,100,100] -> out [400, 96, 96], 5x5 kernel
  mid = (p1 + relu-path)/2; pixel-shuffle (host side, pure data movement)

Mapping to TensorE: contraction K packed as (channel, kh-shift) on partitions
(125/128-wide), kw handled by free-dim offsets into kh-shifted input copies,
accumulated in PSUM across kw / K-chunks.

Sharding: batch (2) x output-channel chunk (4 x 100) = 8 cores. conv2a is
replicated per core (small); gather + pixel shuffle on host.
"""

import os
import time

import ml_dtypes
import numpy as np

import concourse.tile as tile
from concourse import bacc, mybir

F32 = mybir.dt.float32
F32R = mybir.dt.float32r  # single-pass reduced-precision fp32 matmul (4x faster)
BF16 = mybir.dt.bfloat16

# matmul operand dtype: bf16 halves weight-load time (FWL) and data traffic;
# f32r keeps ~19-bit mantissa if more precision is needed.
MM_DT = {"bf16": BF16, "f32r": F32R}[os.environ.get("MM_DT", "bf16")]
MM_NP = {BF16: ml_dtypes.bfloat16, F32R: np.float32}[MM_DT]

B = 2
H = 96
W = 96
HP = H + 8  # 104
WP = W + 8  # 104
CO = 100   # output channels per core (400 / 4)
R = 4      # output rows per block

_RUNNERS = {}


def _build_nc(reps=1):
    nc = bacc.Bacc("TRN2", target_bir_lowering=False, debug=False,
                   enable_asserts=True, num_devices=8)

    xk1 = nc.dram_tensor("xk1", [125, 100, WP], MM_DT, kind="ExternalInput").ap()
    xk2 = nc.dram_tensor("xk2", [100, 96, WP], MM_DT, kind="ExternalInput").ap()
    w1a = nc.dram_tensor("w1a", [125, 9, CO], MM_DT, kind="ExternalInput").ap()
    w1b = nc.dram_tensor("w1b", [100, 9, CO], MM_DT, kind="ExternalInput").ap()
    w2a = nc.dram_tensor("w2a", [125, 5, 180], MM_DT, kind="ExternalInput").ap()
    w2b1 = nc.dram_tensor("w2b1", [128, 25, CO], MM_DT, kind="ExternalInput").ap()
    # conv2b sigma-chunk2 (52 wide) repacked with kh into full-width chunks:
    # t = kh*64 + sigma' (64-padded so partition bases stay 32-aligned),
    # 320 total -> chunks 128/128/64; pad lanes have zero weights
    w2bk = nc.dram_tensor("w2bk", [320, 5, CO], MM_DT, kind="ExternalInput").ap()
    ba1 = nc.dram_tensor("ba1", [128, 1], F32, kind="ExternalInput").ap()
    ba2 = nc.dram_tensor("ba2", [52, 1], F32, kind="ExternalInput").ap()
    b1h = nc.dram_tensor("b1h", [CO, 1], F32, kind="ExternalInput").ap()
    b2bh = nc.dram_tensor("b2bh", [CO, 1], F32, kind="ExternalInput").ap()
    # one output per rep so no rep's work is dead (reps>1 is timing-only)
    outs_d = [nc.dram_tensor("out" if r == 0 else f"out{r}", [CO, H, W], F32,
                             kind="ExternalOutput").ap() for r in range(reps)]

    Relu = mybir.ActivationFunctionType.Relu
    Add = mybir.AluOpType.add

    def mm(out, lhsT, rhs, start, stop):
        nc.tensor.matmul(out, lhsT, rhs, start=start, stop=stop)

    from contextlib import ExitStack

    with tile.TileContext(nc) as tc:
        with (
            tc.tile_pool(name="const", bufs=1) as const,
            tc.tile_pool(name="tmp", bufs=3) as tmp,
            tc.tile_pool(name="outp", bufs=3) as outp,
        ):
          for _rep in range(reps):
            out = outs_d[_rep]
            _ph_a = ExitStack()
            xk1p = _ph_a.enter_context(
                tc.tile_pool(name=f"xk1p{_rep}", bufs=4))
            xk2p = _ph_a.enter_context(
                tc.tile_pool(name=f"xk2p{_rep}", bufs=4))
            psa = _ph_a.enter_context(
                tc.tile_pool(name=f"psa{_rep}", bufs=3, space="PSUM"))
            ps1p = _ph_a.enter_context(
                tc.tile_pool(name=f"ps1{_rep}", bufs=2, space="PSUM"))
            w1a_t = const.tile([125, 9, CO], MM_DT)
            w1b_t = const.tile([100, 9, CO], MM_DT)
            w2a_t = const.tile([125, 5, 180], MM_DT)
            w2b1_t = const.tile([128, 25, CO], MM_DT)
            w2bka_t = const.tile([128, 5, CO], MM_DT)
            w2bkb_t = const.tile([128, 5, CO], MM_DT)
            w2bkc_t = const.tile([64, 5, CO], MM_DT)
            ba1_t = const.tile([128, 1], F32)
            ba2_t = const.tile([52, 1], F32)
            b1h_t = const.tile([CO, 1], F32)
            b2bh_t = const.tile([CO, 1], F32)
            buf1_t = const.tile([128, 100, 100], MM_DT)
            buf2_t = const.tile([52, 100, 100], MM_DT)
            # kh-shifted copies of buf2: bk2[kh*52+s', r, w] = buf2[s', r+kh, w]
            bk2a_t = const.tile([128, H, 100], MM_DT)
            bk2b_t = const.tile([128, H, 100], MM_DT)
            bk2c_t = const.tile([64, H, 100], MM_DT)
            # zero the pad lanes (0 * garbage could be NaN)
            for bt in (bk2a_t, bk2b_t, bk2c_t):
                nc.gpsimd.memset(bt[:], 0.0)
            p1h_t = const.tile([CO, H, W], F32)

            # weights on the gpsimd queue so the streaming xk loads on the
            # sync queue aren't stuck behind 6+ MB of weight traffic
            for t, src in ((w2a_t, w2a), (w1a_t, w1a), (w1b_t, w1b),
                           (w2b1_t, w2b1), (ba1_t, ba1),
                           (w2bka_t, w2bk[0:128]), (w2bkb_t, w2bk[128:256]),
                           (w2bkc_t, w2bk[256:320]),
                           (ba2_t, ba2), (b1h_t, b1h), (b2bh_t, b2bh)):
                nc.gpsimd.dma_start(out=t[:], in_=src)

            # kh -> (dst_tile, dst_off): t = kh*64 + sigma' lands 32-aligned
            bk2_slot = {0: (bk2a_t, 0), 1: (bk2a_t, 64), 2: (bk2b_t, 0),
                        3: (bk2b_t, 64), 4: (bk2c_t, 0)}

            # ---- Phase A: conv2a (25 row blocks) + conv1 (24 row blocks),
            # inputs streamed in 20-row macro-chunks (5 big DMAs per tensor)
            MCR = 20
            for mc in range(5):
                m0 = mc * MCR
                xc1 = xk1p.tile([125, MCR, WP], MM_DT)
                nc.sync.dma_start(out=xc1[:], in_=xk1[:, m0:m0 + MCR, :])
                n2 = min(MCR, H - m0)
                if n2 > 0:
                    xc2 = xk2p.tile([100, MCR, WP], MM_DT)
                    nc.sync.dma_start(out=xc2[:, 0:n2, :],
                                      in_=xk2[:, m0:m0 + n2, :])

                for j in range(5):
                    rc = mc * 5 + j
                    r0 = rc * R       # global output row
                    q0 = j * R        # row within macro-chunk
                    # conv2a: out channels sigma=(a1',a2',c) in two M chunks
                    pa1 = psa.tile([128, R, 100], F32, tag="pa")
                    for kw in range(5):
                        mm(pa1[:], w2a_t[:, kw, 0:128],
                           xc1[:, q0:q0 + R, kw:kw + 100],
                           start=(kw == 0), stop=(kw == 4))
                    nc.scalar.activation(buf1_t[:, r0:r0 + R, :], pa1[:],
                                         Relu, bias=ba1_t[:])
                    pa2 = psa.tile([52, R, 100], F32, tag="pa")
                    for kw in range(5):
                        mm(pa2[:], w2a_t[:, kw, 128:180],
                           xc1[:, q0:q0 + R, kw:kw + 100],
                           start=(kw == 0), stop=(kw == 4))
                    # chunk2 relu on DVE so ACT isn't the drain bottleneck
                    nc.vector.tensor_scalar(buf2_t[:, r0:r0 + R, :], pa2[:],
                                            ba2_t[:], 0.0, Add,
                                            mybir.AluOpType.max)
                    # scatter fresh buf2 rows into their kh-shifted bk2 slots
                    for kh in range(5):
                        lo_r = max(r0 - kh, 0)
                        hi_r = min(r0 + R - kh, H)
                        if hi_r <= lo_r:
                            continue
                        dst, off = bk2_slot[kh]
                        nc.vector.tensor_copy(
                            dst[off:off + 52, lo_r:hi_r, :],
                            buf2_t[:, lo_r + kh:hi_r + kh, :])

                    # conv1 (valid output rows 0..95 only)
                    if rc < 24:
                        p1 = ps1p.tile([CO, R, W], F32)
                        for kw in range(9):
                            mm(p1[:], w1a_t[:, kw, :],
                               xc1[:, q0:q0 + R, kw:kw + W],
                               start=(kw == 0), stop=False)
                        for kw in range(9):
                            mm(p1[:], w1b_t[:, kw, :],
                               xc2[:, q0:q0 + R, kw:kw + W],
                               start=False, stop=(kw == 8))
                        # w1/b1 pre-halved on host: p1h = psum + b1h
                        nc.vector.tensor_scalar_add(p1h_t[:, r0:r0 + R, :],
                                                    p1[:], b1h_t[:])

            # phase-A psum/xk pools released -> conv2b gets 6 PSUM banks
            _ph_a.close()
            ps2p = ExitStack()
            ps2 = ps2p.enter_context(
                tc.tile_pool(name=f"ps2{_rep}", bufs=6, space="PSUM"))

            # ---- Phase B: conv2b (24 row blocks) + merge ----
            for hb in range(24):
                h0 = hb * R
                p2 = ps2.tile([CO, R, W], F32)
                first = True
                for kh in range(5):
                    for kw in range(5):
                        mm(p2[:], w2b1_t[:, kh * 5 + kw, :],
                           buf1_t[:, h0 + kh:h0 + kh + R, kw:kw + W],
                           start=first, stop=False)
                        first = False
                for wt, bt in ((w2bka_t, bk2a_t), (w2bkb_t, bk2b_t),
                               (w2bkc_t, bk2c_t)):
                    for kw in range(5):
                        mm(p2[:], wt[:, kw, :],
                           bt[:, h0:h0 + R, kw:kw + W],
                           start=False,
                           stop=(wt is w2bkc_t and kw == 4))
                # w2b/b2b pre-halved on host: relu(conv2b + b2b)/2 = relu(psum + b2bh)
                tt = tmp.tile([CO, R, W], F32)
                nc.scalar.activation(tt[:], p2[:], Relu, bias=b2bh_t[:])
                ot = outp.tile([CO, R, W], F32)
                nc.vector.tensor_add(ot[:], tt[:], p1h_t[:, h0:h0 + R, :])
                nc.scalar.dma_start(out=out[:, h0:h0 + R, :], in_=ot[:])
            ps2p.close()

    nc.compile()
    return nc


def _w2bk64(W2B):
    """[kh*64+sigma', kw, j] zero-padded repack of W2B[128:]."""
    src = W2B[128:].reshape(52, 5, 5, CO)  # [sigma', kh, kw, j]
    out = np.zeros((320, 5, CO), dtype=np.float32)
    for kh in range(5):
        out[kh * 64:kh * 64 + 52] = src[:, kh, :, :]
    return np.ascontiguousarray(out.astype(MM_NP))


def _prep_in_maps(pic, w1, b1, w2a, b2a, w2b, b2b):
    pic = np.asarray(pic, dtype=np.float32).reshape(B, 25, H, W)
    w1r = np.asarray(w1, dtype=np.float32).reshape(400, 25, 9, 9)
    b1 = np.asarray(b1, dtype=np.float32)
    w2a = np.asarray(w2a, dtype=np.float32)
    b2a = np.asarray(b2a, dtype=np.float32)
    w2b = np.asarray(w2b, dtype=np.float32)
    b2b = np.asarray(b2b, dtype=np.float32)

    xpad = np.full((B, 25, HP, WP), 0.5, dtype=np.float32)
    xpad[:, :, 4:4 + H, 4:4 + W] = pic
    # xk1[b, cin*5+kh, r, w] = xpad[b, cin, r+kh, w]   (kh 0..4, r 0..99)
    xk1 = np.stack([xpad[:, :, kh:kh + 100, :] for kh in range(5)],
                   axis=2).reshape(B, 125, 100, WP)
    # xk2[b, cin*4+kh', h, w] = xpad[b, cin, h+5+kh', w] (kh' 0..3, h 0..95)
    xk2 = np.stack([xpad[:, :, 5 + kh:5 + kh + 96, :] for kh in range(4)],
                   axis=2).reshape(B, 100, 96, WP)

    # W2A[p=(a1*5+a2)*5+kh, kw, m=a1'*60+a2'*20+c] = w2a[c,0,da1,da2,kh,kw]
    W2A = np.zeros((125, 5, 180), dtype=np.float32)
    for a1p in range(3):
        for a2p in range(3):
            m0 = a1p * 60 + a2p * 20
            for da1 in range(3):
                for da2 in range(3):
                    p0 = ((a1p + da1) * 5 + (a2p + da2)) * 5
                    W2A[p0:p0 + 5, :, m0:m0 + 20] = np.transpose(
                        w2a[:, 0, da1, da2, :, :], (1, 2, 0))
    ba_full = np.tile(b2a, 9).astype(np.float32)[:, None]  # [180,1]

    in_maps = []
    for core in range(8):
        b, cc = divmod(core, 4)
        co0 = cc * CO
        # w1, w2b (and their biases) pre-scaled by 0.5 so the (p1+p2)/2
        # average is folded into the matmuls.
        w1sl = 0.5 * w1r[co0:co0 + CO]  # [100, 25, 9, 9]
        W1A = np.ascontiguousarray(
            np.transpose(w1sl[:, :, 0:5, :], (1, 2, 3, 0)).reshape(125, 9, CO))
        W1B = np.ascontiguousarray(
            np.transpose(w1sl[:, :, 5:9, :], (1, 2, 3, 0)).reshape(100, 9, CO))
        w2bsl = 0.5 * w2b[co0:co0 + CO]  # [100, 20, 3, 3, 5, 5]
        W2B = np.ascontiguousarray(
            np.transpose(w2bsl, (2, 3, 1, 4, 5, 0)).reshape(180, 25, CO))
        in_maps.append({
            "xk1": np.ascontiguousarray(xk1[b].astype(MM_NP)),
            "xk2": np.ascontiguousarray(xk2[b].astype(MM_NP)),
            "w1a": W1A.astype(MM_NP),
            "w1b": W1B.astype(MM_NP),
            "w2a": W2A.astype(MM_NP),
            "w2b1": np.ascontiguousarray(W2B[:128].astype(MM_NP)),
            # sigma-chunk2 repacked as t = kh*64 + sigma' (zero-padded)
            "w2bk": _w2bk64(W2B),
            "ba1": np.ascontiguousarray(ba_full[:128]),
            "ba2": np.ascontiguousarray(ba_full[128:]),
            "b1h": np.ascontiguousarray((0.5 * b1[co0:co0 + CO])[:, None]),
            "b2bh": np.ascontiguousarray((0.5 * b2b[co0:co0 + CO])[:, None]),
        })
    return in_maps


def _get_runner(reps=1):
    """Build nc once per reps and return a cached jitted SPMD executor."""
    if reps in _RUNNERS:
        return _RUNNERS[reps]

    import jax
    from jax.experimental.shard_map import shard_map
    from jax.sharding import Mesh, NamedSharding, PartitionSpec

    from concourse import mybir as _mybir
    from concourse.bass2jax import (_bass_exec_p, install_neuronx_cc_hook,
                                    partition_id_tensor)

    nc = _build_nc(reps)
    install_neuronx_cc_hook()

    n_cores = 8
    partition_name = (nc.partition_id_tensor.name
                      if nc.partition_id_tensor else None)
    in_names, out_names, out_avals, zero_outs = [], [], [], []
    for alloc in nc.m.functions[0].allocations:
        if not isinstance(alloc, _mybir.MemoryLocationSet):
            continue
        name = alloc.memorylocations[0].name
        if alloc.kind == "ExternalInput":
            if name != partition_name:
                in_names.append(name)
        elif alloc.kind == "ExternalOutput":
            shape = tuple(alloc.tensor_shape)
            dtype = _mybir.dt.np(alloc.dtype)
            out_names.append(name)
            out_avals.append(jax.core.ShapedArray(shape, dtype))
            zero_outs.append(np.zeros((n_cores * shape[0],) + shape[1:], dtype))
    assert nc.dbg_addr is None
    n_params = len(in_names)
    all_names = in_names + out_names
    if partition_name is not None:
        all_names = all_names + [partition_name]
    donate = tuple(range(n_params, n_params + len(out_names)))

    def _body(*args):
        operands = list(args)
        if partition_name is not None:
            operands.append(partition_id_tensor())
        outs = _bass_exec_p.bind(
            *operands,
            out_avals=tuple(out_avals),
            in_names=tuple(all_names),
            out_names=tuple(out_names),
            lowering_input_output_aliases=(),
            sim_require_finite=True,
            sim_require_nnan=True,
            nc=nc,
        )
        return tuple(outs)

    devices = jax.devices()[:n_cores]
    mesh = Mesh(np.asarray(devices), ("core",))
    nspec = (PartitionSpec("core"),) * (n_params + len(out_names))
    sharded = jax.jit(
        shard_map(_body, mesh=mesh, in_specs=nspec,
                  out_specs=(PartitionSpec("core"),) * len(out_names)),
        keep_unused=True)
    sharding = NamedSharding(mesh, PartitionSpec("core"))

    class Runner:
        def put(self, in_maps):
            """Transfer inputs (+ zero output bufs) to the devices once."""
            concat_in = [
                np.concatenate([np.asarray(m[name]) for m in in_maps], axis=0)
                for name in in_names
            ]
            return [jax.device_put(x, sharding)
                    for x in concat_in + zero_outs]

        def exec_timed(self, dev_args):
            t0 = time.perf_counter()
            out_arrs = sharded(*dev_args)
            # one sync only: under axon each block_until_ready is a costly
            # RPC, and blocking any output waits for the whole execution
            out_arrs[0].block_until_ready()
            return out_arrs, time.perf_counter() - t0

        def __call__(self, in_maps):
            out_arrs, dt = self.exec_timed(self.put(in_maps))
            per_core = [
                {name: np.asarray(out_arrs[i]).reshape(
                    n_cores, *out_avals[i].shape)[c]
                 for i, name in enumerate(out_names)}
                for c in range(n_cores)
            ]
            return per_core, dt

    run = Runner()
    _RUNNERS[reps] = run
    return run


def kernel(pic, w1, b1, w2a, b2a, w2b, b2b):
    run = _get_runner()
    in_maps = _prep_in_maps(pic, w1, b1, w2a, b2a, w2b, b2b)
    results, _ = run(in_maps)

    mid = np.empty((B, 400, H, W), dtype=np.float32)
    for core in range(8):
        b, cc = divmod(core, 4)
        mid[b, cc * CO:(cc + 1) * CO] = results[core]["out"]
    # pixel shuffle r=4, then split 25 -> 5x5
    y = mid.reshape(B, 25, 4, 4, H, W).transpose(0, 1, 4, 2, 5, 3)
    return np.ascontiguousarray(y).reshape(B, 5, 5, H * 4, W * 4)


# revision 60
# speedup vs baseline: 1.2341x; 1.2341x over previous
"""Trainium2 Bass kernel for nn_DimNet (4D-conv net + pixel shuffle).

Math: the three 4D convs collapse to 2D convs over flattened angular dims:
  conv1:  in [25, 104, 104] -> out [400, 96, 96], 9x9 kernel
  conv2a: in [25, 104, 104] -> buf [180, 100, 100] (20ch x 3x3 angular window, 5x5)
  conv2b: buf [180,100,100] -> out [400, 96, 96], 5x5 kernel
  mid = (p1 + relu-path)/2; pixel-shuffle (host side, pure data movement)

Mapping to TensorE: contraction K packed as (channel, kh-shift) on partitions
(125/128-wide), kw handled by free-dim offsets into kh-shifted input copies,
accumulated in PSUM across kw / K-chunks.

Sharding: batch (2) x output-channel chunk (4 x 100) = 8 cores. conv2a is
replicated per core (small); gather + pixel shuffle on host.
"""

import os
import time

import ml_dtypes
import numpy as np

import concourse.tile as tile
from concourse import bacc, mybir

F32 = mybir.dt.float32
F32R = mybir.dt.float32r  # single-pass reduced-precision fp32 matmul (4x faster)
BF16 = mybir.dt.bfloat16

# matmul operand dtype: bf16 halves weight-load time (FWL) and data traffic;
# f32r keeps ~19-bit mantissa if more precision is needed.
MM_DT = {"bf16": BF16, "f32r": F32R}[os.environ.get("MM_DT", "bf16")]
MM_NP = {BF16: ml_dtypes.bfloat16, F32R: np.float32}[MM_DT]

B = 2
H = 96
W = 96
HP = H + 8  # 104
WP = W + 8  # 104
CO = 100   # output channels per core (400 / 4)

_RUNNERS = {}


def _build_nc(reps=1):
    nc = bacc.Bacc("TRN2", target_bir_lowering=False, debug=False,
                   enable_asserts=True, num_devices=8)

    xk1 = nc.dram_tensor("xk1", [125, 100, WP], MM_DT, kind="ExternalInput").ap()
    xk2 = nc.dram_tensor("xk2", [100, 96, WP], MM_DT, kind="ExternalInput").ap()
    w1a = nc.dram_tensor("w1a", [125, 9, CO], MM_DT, kind="ExternalInput").ap()
    w1b = nc.dram_tensor("w1b", [100, 9, CO], MM_DT, kind="ExternalInput").ap()
    w2a = nc.dram_tensor("w2a", [125, 5, 180], MM_DT, kind="ExternalInput").ap()
    w2b1 = nc.dram_tensor("w2b1", [128, 25, CO], MM_DT, kind="ExternalInput").ap()
    # conv2b sigma-chunk2 (52 wide) repacked with kh into full-width chunks:
    # t = kh*64 + sigma' (64-padded so partition bases stay 32-aligned),
    # 320 total -> chunks 128/128/64; pad lanes have zero weights
    w2bk = nc.dram_tensor("w2bk", [320, 5, CO], MM_DT, kind="ExternalInput").ap()
    ba1 = nc.dram_tensor("ba1", [128, 1], F32, kind="ExternalInput").ap()
    ba2 = nc.dram_tensor("ba2", [52, 1], F32, kind="ExternalInput").ap()
    b1h = nc.dram_tensor("b1h", [CO, 1], F32, kind="ExternalInput").ap()
    b2bh = nc.dram_tensor("b2bh", [CO, 1], F32, kind="ExternalInput").ap()
    # one output per rep so no rep's work is dead (reps>1 is timing-only)
    outs_d = [nc.dram_tensor("out" if r == 0 else f"out{r}", [CO, H, W], F32,
                             kind="ExternalOutput").ap() for r in range(reps)]

    Relu = mybir.ActivationFunctionType.Relu
    Add = mybir.AluOpType.add

    def mm(out, lhsT, rhs, start, stop):
        nc.tensor.matmul(out, lhsT, rhs, start=start, stop=stop)

    from contextlib import ExitStack

    with tile.TileContext(nc) as tc:
        with (
            tc.tile_pool(name="const", bufs=1) as const,
            tc.tile_pool(name="tmp", bufs=3) as tmp,
            tc.tile_pool(name="outp", bufs=3) as outp,
        ):
          # weights/biases loaded once (shared across timing reps)
          w1a_t = const.tile([125, 9, CO], MM_DT)
          w1b_t = const.tile([100, 9, CO], MM_DT)
          w2a_t = const.tile([125, 5, 180], MM_DT)
          w2b1_t = const.tile([128, 25, CO], MM_DT)
          w2bka_t = const.tile([128, 5, CO], MM_DT)
          w2bkb_t = const.tile([128, 5, CO], MM_DT)
          w2bkc_t = const.tile([64, 5, CO], MM_DT)
          ba1_t = const.tile([128, 1], F32)
          ba2_t = const.tile([52, 1], F32)
          b1h_t = const.tile([CO, 1], F32)
          b2bh_t = const.tile([CO, 1], F32)
          # weights on the gpsimd queue so the streaming xk loads on the
          # sync queue aren't stuck behind 6+ MB of weight traffic
          for t, src in ((w2a_t, w2a), (w1a_t, w1a), (w1b_t, w1b),
                         (w2b1_t, w2b1), (ba1_t, ba1),
                         (w2bka_t, w2bk[0:128]), (w2bkb_t, w2bk[128:256]),
                         (w2bkc_t, w2bk[256:320]),
                         (ba2_t, ba2), (b1h_t, b1h), (b2bh_t, b2bh)):
              nc.gpsimd.dma_start(out=t[:], in_=src)

          # kh-shifted copies of buf2: bk2[kh*64+s', r, w] = buf2[s', r+kh, w]
          # (persistent; pad lanes zeroed once -- 0 * garbage could be NaN)
          bk2a_t = const.tile([128, H, 100], MM_DT)
          bk2b_t = const.tile([128, H, 100], MM_DT)
          bk2c_t = const.tile([64, H, 100], MM_DT)
          for bt in (bk2a_t, bk2b_t, bk2c_t):
              nc.gpsimd.memset(bt[:], 0.0)

          for _rep in range(reps):
            out = outs_d[_rep]
            _ph_a = ExitStack()
            xk1p = _ph_a.enter_context(
                tc.tile_pool(name=f"xk1p{_rep}", bufs=4))
            xk2p = _ph_a.enter_context(
                tc.tile_pool(name=f"xk2p{_rep}", bufs=4))
            psa = _ph_a.enter_context(
                tc.tile_pool(name=f"psa{_rep}", bufs=3, space="PSUM"))
            ps1p = _ph_a.enter_context(
                tc.tile_pool(name=f"ps1{_rep}", bufs=2, space="PSUM"))
            buf1_t = const.tile([128, 100, 100], MM_DT)
            buf2_t = const.tile([52, 100, 100], MM_DT)
            p1h_t = const.tile([CO, H, W], F32)

            # kh -> (dst_tile, dst_off): t = kh*64 + sigma' lands 32-aligned
            bk2_slot = {0: (bk2a_t, 0), 1: (bk2a_t, 64), 2: (bk2b_t, 0),
                        3: (bk2b_t, 64), 4: (bk2c_t, 0)}

            # ---- Phase A: conv2a (20 5-row blocks) + conv1 (19 5-row + 1),
            # inputs streamed in 20-row macro-chunks (5 big DMAs per tensor)
            MCR = 20
            RB = 5
            for mc in range(5):
                m0 = mc * MCR
                xc1 = xk1p.tile([125, MCR, WP], MM_DT)
                nc.sync.dma_start(out=xc1[:], in_=xk1[:, m0:m0 + MCR, :])
                n2 = min(MCR, H - m0)
                if n2 > 0:
                    xc2 = xk2p.tile([100, MCR, WP], MM_DT)
                    nc.sync.dma_start(out=xc2[:, 0:n2, :],
                                      in_=xk2[:, m0:m0 + n2, :])

                for j in range(4):
                    r0 = m0 + j * RB  # global row
                    q0 = j * RB       # row within macro-chunk
                    # conv2a: out channels sigma=(a1',a2',c) in two M chunks
                    pa1 = psa.tile([128, RB, 100], F32, tag="pa")
                    for kw in range(5):
                        mm(pa1[:], w2a_t[:, kw, 0:128],
                           xc1[:, q0:q0 + RB, kw:kw + 100],
                           start=(kw == 0), stop=(kw == 4))
                    nc.scalar.activation(buf1_t[:, r0:r0 + RB, :], pa1[:],
                                         Relu, bias=ba1_t[:])
                    pa2 = psa.tile([52, RB, 100], F32, tag="pa")
                    for kw in range(5):
                        mm(pa2[:], w2a_t[:, kw, 128:180],
                           xc1[:, q0:q0 + RB, kw:kw + 100],
                           start=(kw == 0), stop=(kw == 4))
                    # chunk2 relu on DVE so ACT isn't the drain bottleneck
                    nc.vector.tensor_scalar(buf2_t[:, r0:r0 + RB, :], pa2[:],
                                            ba2_t[:], 0.0, Add,
                                            mybir.AluOpType.max)
                    # scatter fresh buf2 rows into their kh-shifted bk2 slots
                    for kh in range(5):
                        lo_r = max(r0 - kh, 0)
                        hi_r = min(r0 + RB - kh, H)
                        if hi_r <= lo_r:
                            continue
                        dst, off = bk2_slot[kh]
                        nc.vector.tensor_copy(
                            dst[off:off + 52, lo_r:hi_r, :],
                            buf2_t[:, lo_r + kh:hi_r + kh, :])

                    # conv1 on the same 5-row grid (rows 0..94; row 95 below)
                    rr = min(RB, H - r0)
                    if rr > 0:
                        p1 = ps1p.tile([CO, RB, W], F32)
                        for kw in range(9):
                            mm(p1[:, 0:rr, :], w1a_t[:, kw, :],
                               xc1[:, q0:q0 + rr, kw:kw + W],
                               start=(kw == 0), stop=False)
                        for kw in range(9):
                            mm(p1[:, 0:rr, :], w1b_t[:, kw, :],
                               xc2[:, q0:q0 + rr, kw:kw + W],
                               start=False, stop=(kw == 8))
                        # w1/b1 pre-halved on host: p1h = psum + b1h
                        nc.vector.tensor_scalar_add(p1h_t[:, r0:r0 + rr, :],
                                                    p1[:, 0:rr, :], b1h_t[:])

            # phase-A psum/xk pools released -> conv2b gets 6 PSUM banks
            _ph_a.close()
            ps2p = ExitStack()
            ps2 = ps2p.enter_context(
                tc.tile_pool(name=f"ps2{_rep}", bufs=6, space="PSUM"))

            # ---- Phase B: conv2b (19 5-row blocks + 1 single-row) + merge ----
            for h0 in list(range(0, 95, RB)) + [95]:
                rr = min(RB, H - h0)
                p2 = ps2.tile([CO, RB, W], F32)
                p2v = p2[:, 0:rr, :]
                first = True
                for kh in range(5):
                    for kw in range(5):
                        mm(p2v, w2b1_t[:, kh * 5 + kw, :],
                           buf1_t[:, h0 + kh:h0 + kh + rr, kw:kw + W],
                           start=first, stop=False)
                        first = False
                for wt, bt in ((w2bka_t, bk2a_t), (w2bkb_t, bk2b_t),
                               (w2bkc_t, bk2c_t)):
                    for kw in range(5):
                        mm(p2v, wt[:, kw, :],
                           bt[:, h0:h0 + rr, kw:kw + W],
                           start=False,
                           stop=(wt is w2bkc_t and kw == 4))
                # w2b/b2b pre-halved on host: relu(conv2b+b2b)/2 = relu(psum+b2bh)
                tt = tmp.tile([CO, RB, W], F32)
                nc.scalar.activation(tt[:, 0:rr, :], p2v, Relu, bias=b2bh_t[:])
                ot = outp.tile([CO, RB, W], F32)
                nc.vector.tensor_add(ot[:, 0:rr, :], tt[:, 0:rr, :],
                                     p1h_t[:, h0:h0 + rr, :])
                nc.scalar.dma_start(out=out[:, h0:h0 + rr, :],
                                    in_=ot[:, 0:rr, :])
            ps2p.close()

    nc.compile()
    return nc


def _w2bk64(W2B):
    """[kh*64+sigma', kw, j] zero-padded repack of W2B[128:]."""
    src = W2B[128:].reshape(52, 5, 5, CO)  # [sigma', kh, kw, j]
    out = np.zeros((320, 5, CO), dtype=np.float32)
    for kh in range(5):
        out[kh * 64:kh * 64 + 52] = src[:, kh, :, :]
    return np.ascontiguousarray(out.astype(MM_NP))


def _prep_in_maps(pic, w1, b1, w2a, b2a, w2b, b2b):
    pic = np.asarray(pic, dtype=np.float32).reshape(B, 25, H, W)
    w1r = np.asarray(w1, dtype=np.float32).reshape(400, 25, 9, 9)
    b1 = np.asarray(b1, dtype=np.float32)
    w2a = np.asarray(w2a, dtype=np.float32)
    b2a = np.asarray(b2a, dtype=np.float32)
    w2b = np.asarray(w2b, dtype=np.float32)
    b2b = np.asarray(b2b, dtype=np.float32)

    xpad = np.full((B, 25, HP, WP), 0.5, dtype=np.float32)
    xpad[:, :, 4:4 + H, 4:4 + W] = pic
    # xk1[b, cin*5+kh, r, w] = xpad[b, cin, r+kh, w]   (kh 0..4, r 0..99)
    xk1 = np.stack([xpad[:, :, kh:kh + 100, :] for kh in range(5)],
                   axis=2).reshape(B, 125, 100, WP)
    # xk2[b, cin*4+kh', h, w] = xpad[b, cin, h+5+kh', w] (kh' 0..3, h 0..95)
    xk2 = np.stack([xpad[:, :, 5 + kh:5 + kh + 96, :] for kh in range(4)],
                   axis=2).reshape(B, 100, 96, WP)

    # W2A[p=(a1*5+a2)*5+kh, kw, m=a1'*60+a2'*20+c] = w2a[c,0,da1,da2,kh,kw]
    W2A = np.zeros((125, 5, 180), dtype=np.float32)
    for a1p in range(3):
        for a2p in range(3):
            m0 = a1p * 60 + a2p * 20
            for da1 in range(3):
                for da2 in range(3):
                    p0 = ((a1p + da1) * 5 + (a2p + da2)) * 5
                    W2A[p0:p0 + 5, :, m0:m0 + 20] = np.transpose(
                        w2a[:, 0, da1, da2, :, :], (1, 2, 0))
    ba_full = np.tile(b2a, 9).astype(np.float32)[:, None]  # [180,1]

    in_maps = []
    for core in range(8):
        b, cc = divmod(core, 4)
        co0 = cc * CO
        # w1, w2b (and their biases) pre-scaled by 0.5 so the (p1+p2)/2
        # average is folded into the matmuls.
        w1sl = 0.5 * w1r[co0:co0 + CO]  # [100, 25, 9, 9]
        W1A = np.ascontiguousarray(
            np.transpose(w1sl[:, :, 0:5, :], (1, 2, 3, 0)).reshape(125, 9, CO))
        W1B = np.ascontiguousarray(
            np.transpose(w1sl[:, :, 5:9, :], (1, 2, 3, 0)).reshape(100, 9, CO))
        w2bsl = 0.5 * w2b[co0:co0 + CO]  # [100, 20, 3, 3, 5, 5]
        W2B = np.ascontiguousarray(
            np.transpose(w2bsl, (2, 3, 1, 4, 5, 0)).reshape(180, 25, CO))
        in_maps.append({
            "xk1": np.ascontiguousarray(xk1[b].astype(MM_NP)),
            "xk2": np.ascontiguousarray(xk2[b].astype(MM_NP)),
            "w1a": W1A.astype(MM_NP),
            "w1b": W1B.astype(MM_NP),
            "w2a": W2A.astype(MM_NP),
            "w2b1": np.ascontiguousarray(W2B[:128].astype(MM_NP)),
            # sigma-chunk2 repacked as t = kh*64 + sigma' (zero-padded)
            "w2bk": _w2bk64(W2B),
            "ba1": np.ascontiguousarray(ba_full[:128]),
            "ba2": np.ascontiguousarray(ba_full[128:]),
            "b1h": np.ascontiguousarray((0.5 * b1[co0:co0 + CO])[:, None]),
            "b2bh": np.ascontiguousarray((0.5 * b2b[co0:co0 + CO])[:, None]),
        })
    return in_maps


def _get_runner(reps=1):
    """Build nc once per reps and return a cached jitted SPMD executor."""
    if reps in _RUNNERS:
        return _RUNNERS[reps]

    import jax
    from jax.experimental.shard_map import shard_map
    from jax.sharding import Mesh, NamedSharding, PartitionSpec

    from concourse import mybir as _mybir
    from concourse.bass2jax import (_bass_exec_p, install_neuronx_cc_hook,
                                    partition_id_tensor)

    nc = _build_nc(reps)
    install_neuronx_cc_hook()

    n_cores = 8
    partition_name = (nc.partition_id_tensor.name
                      if nc.partition_id_tensor else None)
    in_names, out_names, out_avals, zero_outs = [], [], [], []
    for alloc in nc.m.functions[0].allocations:
        if not isinstance(alloc, _mybir.MemoryLocationSet):
            continue
        name = alloc.memorylocations[0].name
        if alloc.kind == "ExternalInput":
            if name != partition_name:
                in_names.append(name)
        elif alloc.kind == "ExternalOutput":
            shape = tuple(alloc.tensor_shape)
            dtype = _mybir.dt.np(alloc.dtype)
            out_names.append(name)
            out_avals.append(jax.core.ShapedArray(shape, dtype))
            zero_outs.append(np.zeros((n_cores * shape[0],) + shape[1:], dtype))
    assert nc.dbg_addr is None
    n_params = len(in_names)
    all_names = in_names + out_names
    if partition_name is not None:
        all_names = all_names + [partition_name]

    def _body(*args):
        operands = list(args)
        if partition_name is not None:
            operands.append(partition_id_tensor())
        outs = _bass_exec_p.bind(
            *operands,
            out_avals=tuple(out_avals),
            in_names=tuple(all_names),
            out_names=tuple(out_names),
            lowering_input_output_aliases=(),
            sim_require_finite=True,
            sim_require_nnan=True,
            nc=nc,
        )
        return tuple(outs)

    devices = jax.devices()[:n_cores]
    mesh = Mesh(np.asarray(devices), ("core",))
    nspec = (PartitionSpec("core"),) * (n_params + len(out_names))
    sharded = jax.jit(
        shard_map(_body, mesh=mesh, in_specs=nspec,
                  out_specs=(PartitionSpec("core"),) * len(out_names)),
        keep_unused=True)
    sharding = NamedSharding(mesh, PartitionSpec("core"))

    class Runner:
        def put(self, in_maps):
            """Transfer inputs (+ zero output bufs) to the devices once."""
            concat_in = [
                np.concatenate([np.asarray(m[name]) for m in in_maps], axis=0)
                for name in in_names
            ]
            return [jax.device_put(x, sharding)
                    for x in concat_in + zero_outs]

        def exec_timed(self, dev_args):
            t0 = time.perf_counter()
            out_arrs = sharded(*dev_args)
            # one sync only: under axon each block_until_ready is a costly
            # RPC, and blocking any output waits for the whole execution
            out_arrs[0].block_until_ready()
            return out_arrs, time.perf_counter() - t0

        def __call__(self, in_maps):
            out_arrs, dt = self.exec_timed(self.put(in_maps))
            per_core = [
                {name: np.asarray(out_arrs[i]).reshape(
                    n_cores, *out_avals[i].shape)[c]
                 for i, name in enumerate(out_names)}
                for c in range(n_cores)
            ]
            return per_core, dt

    run = Runner()
    _RUNNERS[reps] = run
    return run


def kernel(pic, w1, b1, w2a, b2a, w2b, b2b):
    run = _get_runner()
    in_maps = _prep_in_maps(pic, w1, b1, w2a, b2a, w2b, b2b)
    results, _ = run(in_maps)

    mid = np.empty((B, 400, H, W), dtype=np.float32)
    for core in range(8):
        b, cc = divmod(core, 4)
        mid[b, cc * CO:(cc + 1) * CO] = results[core]["out"]
    # pixel shuffle r=4, then split 25 -> 5x5
    y = mid.reshape(B, 25, 4, 4, H, W).transpose(0, 1, 4, 2, 5, 3)
    return np.ascontiguousarray(y).reshape(B, 5, 5, H * 4, W * 4)
